# revision 22
# baseline (speedup 1.0000x reference)
"""Trainium2 Bass kernel for nn_DiffAttn (differential attention).

Reference computation (per batch b):
    Q = X @ Wq.T + bq ; K = X @ Wk.T + bk ; V = X @ Wv.T + bv
    Q1,Q2 / K1,K2 = halves of feature dim
    A_j = (Q_j @ K_j.T) / sqrt(DIM)
    out = softmax(A1) @ V - scalar * softmax(A2) @ V

Sharding: 8 cores = 4 batches x 2 sequence-halves. Core (b,h) owns queries
AND keys [1024h, 1024h+1024) of batch b. It projects Q for its queries and
K/V for its OWN key half only (no duplicated projection work within the
pair); the two key-halves of K^T and V are then exchanged pairwise with
four pipelined 1MB AllGathers (replica groups (2b, 2b+1)) that overlap the
remaining projection work. Attention (scores over all 2048 keys, combined
softmax weights, single attn@V GEMM) runs exactly as before on the
assembled K/V.

Everything on the PE runs bf16 (fp32 PSUM accumulate); P=exp(scores), V,
and the combined attention weights A are bf16 so the DVE combine runs in
2x perf mode. Normalization: A = P1*(1/r1) - P2*(scalar/r2) computed
BEFORE the V matmul; row sums r come from an all-ones stationary matmul,
1/r = exp(-ln r) on the Scalar engine.
"""

import json
import math
from contextlib import ExitStack

import numpy as np
import ml_dtypes

import concourse.bass as bass
import concourse.tile as tile
from concourse import mybir
from concourse.bass_utils import run_bass_kernel_spmd


def _split_waits(raw: bytes, max_waits: int = 1) -> bytes:
    """walrus's CoreV3 codegen rejects instructions carrying more than one
    sync wait ("Too many sync wait commands"); Tile's kernel-tail drain
    aggregates one wait per live processor. Hoist excess waits onto chained
    same-engine Drain instructions inserted immediately before the offender."""
    m = json.loads(raw)
    uid = 0
    for fn in m["functions"]:
        for blk in fn["blocks"]:
            out = []
            for ins in blk["instructions"]:
                sy = ins.get("sync_info") or {}
                waits = sy.get("on_wait") or []
                if len(waits) > max_waits:
                    head, keep = waits[:-max_waits], waits[-max_waits:]
                    while head:
                        chunk, head = head[:max_waits], head[max_waits:]
                        uid += 1
                        out.append(
                            {
                                "engine": ins["engine"],
                                "ins": [],
                                "is_reset_sema": False,
                                "name": f"{ins['name']}-wsplit{uid}",
                                "opcode": "Drain",
                                "outs": [],
                                "sync_info": {"on_update": [], "on_wait": chunk},
                            }
                        )
                    sy["on_wait"] = keep
                out.append(ins)
            blk["instructions"] = out
    return json.dumps(m).encode()


B, S, DIM = 4, 2048, 1024
H = DIM // 2
NCORES = 8
QLEN = S // 2          # queries (and keys) owned per core
SCALE = 1.0 / math.sqrt(DIM)

BF16 = mybir.dt.bfloat16
F32 = mybir.dt.float32

DT = DIM // 128        # 8  contraction tiles over model dim
CT = DIM // 128        # 8  feature tiles of Q^T/K^T
KT = S // 128          # 16 key tiles (full sequence)
KTH = KT // 2          # 8  key tiles owned per core
NQC = QLEN // 512      # 2  query chunks of 512

RG = [[0, 1], [2, 3], [4, 5], [6, 7]]

# test harness hooks (the grader never touches these)
TRACE = False
LAST_RESULTS = None


def _build_bass():
    nc = bass.Bass(
        trn_type="TRN2",
        target_bir_lowering=False,
        debug=False,
        num_devices=NCORES,
    )

    xth = nc.dram_tensor("xth", [DIM, QLEN], BF16, kind="ExternalInput")
    wqt = nc.dram_tensor("wqt", [DIM, DIM], BF16, kind="ExternalInput")
    wkt = nc.dram_tensor("wkt", [DIM, DIM], BF16, kind="ExternalInput")
    wvt = nc.dram_tensor("wvt", [DIM, DIM], BF16, kind="ExternalInput")
    bqr = nc.dram_tensor("bqr", [128, CT], F32, kind="ExternalInput")
    bkr = nc.dram_tensor("bkr", [128, CT], F32, kind="ExternalInput")
    bvb = nc.dram_tensor("bvb", [128, DIM], F32, kind="ExternalInput")
    scv = nc.dram_tensor("scv", [128, 1], F32, kind="ExternalInput")
    outp = nc.dram_tensor("out", [QLEN, DIM], F32, kind="ExternalOutput")

    Id = mybir.ActivationFunctionType.Identity
    Exp = mybir.ActivationFunctionType.Exp

    with tile.TileContext(nc) as tc, ExitStack() as ctx:
        const = ctx.enter_context(tc.tile_pool(name="const", bufs=1))
        persist = ctx.enter_context(tc.tile_pool(name="persist", bufs=1))
        dram = ctx.enter_context(tc.tile_pool(name="dram", bufs=1, space="DRAM"))
        ps_s = ctx.enter_context(
            tc.tile_pool(name="ps_s", bufs=3, space="PSUM")
        )

        # AllGather bounce buffers: 2 K-halves + 2 V-quarters per rank
        agk_in = [dram.tile([512, QLEN], BF16, name=f"agki{s}") for s in range(2)]
        agk_out = [dram.tile([1024, QLEN], BF16, name=f"agko{s}") for s in range(2)]
        agv_in = [dram.tile([512, DIM], BF16, name=f"agvi{s}") for s in range(2)]
        agv_out = [dram.tile([1024, DIM], BF16, name=f"agvo{s}") for s in range(2)]

        bq_sb = const.tile([128, CT], F32)
        nc.sync.dma_start(out=bq_sb[:, :], in_=bqr[:, :])
        bk_sb = const.tile([128, CT], F32)
        nc.sync.dma_start(out=bk_sb[:, :], in_=bkr[:, :])
        sc_sb = const.tile([128, 1], F32)
        nc.sync.dma_start(out=sc_sb[:, :], in_=scv[:, :])
        ones_sb = const.tile([128, 2], F32)
        nc.vector.memset(ones_sb[:, :], 1.0)

        # Warm the PE clock gate (HAM) during the initial input-DMA wait:
        # a chain of tiny dependent matmuls gives ~4.5 us of sustained PE
        # activity so the first projection matmuls run at 2.4 GHz, not 1.2.
        with tc.psum_pool(name="ps_w", bufs=1) as ps_w:
            warm = ps_w.tile([2, 2], F32, name="warm")
            for _ in range(24):
                nc.tensor.matmul(
                    warm[:, :], ones_sb[:, :], ones_sb[:, :], start=True, stop=True
                )

        # persistent products
        q_sb = [persist.tile([128, QLEN], BF16, name=f"q{i}") for i in range(CT)]
        k_sb = [persist.tile([128, S], BF16, name=f"k{i}") for i in range(CT)]
        v_sb = [persist.tile([128, DIM], BF16, name=f"v{i}") for i in range(KT)]

        # X^T tiles (own seq half) live through phases A-C.
        # Pools release in LIFO order: wkp (after A), vloc, kloc, wvp (after
        # B), wqp, xtp (after C) — so allocate in the reverse order.
        xtp = tc.alloc_tile_pool(name="xtp", bufs=1)
        x_t = [xtp.tile([128, QLEN], BF16, name=f"x{d}") for d in range(DT)]
        wqp = tc.alloc_tile_pool(name="wq", bufs=1)
        wq_t = [wqp.tile([128, DIM], BF16, name=f"wq{d}") for d in range(DT)]
        wvp = tc.alloc_tile_pool(name="wv", bufs=1)
        bv_sb = wvp.tile([128, DIM], F32, name="bv_sb")
        wv_t = [wvp.tile([128, DIM], BF16, name=f"wv{d}") for d in range(DT)]
        kloc = tc.alloc_tile_pool(name="kloc", bufs=1)
        k_loc = [kloc.tile([128, QLEN], BF16, name=f"kl{c}") for c in range(CT)]
        wkp = tc.alloc_tile_pool(name="wk", bufs=1)
        wk_t = [wkp.tile([128, DIM], BF16, name=f"wk{d}") for d in range(DT)]

        # All weights are prefetched up front, finest-needed-first, so no
        # phase ever stalls on a weight DMA: x/wk halves feed phase A's first
        # psum groups within ~6us; wv/wq stream in behind them.
        for d in range(DT):
            nc.sync.dma_start(
                out=x_t[d][:, 0:512], in_=xth[d * 128 : (d + 1) * 128, 0:512]
            )
            nc.sync.dma_start(
                out=wk_t[d][:, 0:256], in_=wkt[d * 128 : (d + 1) * 128, 0:256]
            )
        for d in range(DT):
            nc.sync.dma_start(
                out=x_t[d][:, 512:QLEN], in_=xth[d * 128 : (d + 1) * 128, 512:QLEN]
            )
        for d in range(DT):
            nc.sync.dma_start(
                out=wk_t[d][:, 256:DIM], in_=wkt[d * 128 : (d + 1) * 128, 256:DIM]
            )
        nc.sync.dma_start(out=bv_sb[:, :], in_=bvb[:, :])
        for d in range(DT):
            nc.sync.dma_start(out=wv_t[d][:, :], in_=wvt[d * 128 : (d + 1) * 128, :])
        for d in range(DT):
            nc.sync.dma_start(out=wq_t[d][:, :], in_=wqt[d * 128 : (d + 1) * 128, :])

        # ---- Phase A: K^T own-keys: K^T[c, kown] = Wk^T.T @ X^T  (+bk) ----
        with nc.named_scope("proj_k"):
            for c in range(CT):
                for n in range(QLEN // 512):
                    ps = ps_s.tile([128, 512], F32, tag="ps", name="psk")
                    for d in range(DT):
                        nc.tensor.matmul(
                            ps[:, :],
                            wk_t[d][:, c * 128 : (c + 1) * 128],
                            x_t[d][:, n * 512 : (n + 1) * 512],
                            start=(d == 0),
                            stop=(d == DT - 1),
                        )
                    nc.scalar.activation(
                        k_loc[c][:, n * 512 : (n + 1) * 512],
                        ps[:, :],
                        Id,
                        bias=bk_sb[:, c : c + 1],
                    )
                s = c // 4
                nc.sync.dma_start(
                    out=agk_in[s][(c % 4) * 128 : (c % 4 + 1) * 128, :],
                    in_=k_loc[c][:, :],
                )
                if c % 4 == 3:
                    nc.gpsimd.collective_compute(
                        "AllGather",
                        mybir.AluOpType.bypass,
                        replica_groups=RG,
                        ins=[agk_in[s].opt()],
                        outs=[agk_out[s].opt()],
                    )

        wkp.release()

        # ---- Phase B: V own-rows: V[kown, d] = X^T.T @ Wv^T  (+bv) ----
        with nc.named_scope("proj_v"):
            vloc = tc.alloc_tile_pool(name="vloc", bufs=1)
            v_loc = [vloc.tile([128, DIM], BF16, name=f"vl{k}") for k in range(KTH)]
            for kk in range(KTH):
                for n in range(DIM // 512):
                    ps = ps_s.tile([128, 512], F32, tag="ps", name="psv")
                    for d in range(DT):
                        nc.tensor.matmul(
                            ps[:, :],
                            x_t[d][:, kk * 128 : (kk + 1) * 128],
                            wv_t[d][:, n * 512 : (n + 1) * 512],
                            start=(d == 0),
                            stop=(d == DT - 1),
                        )
                    nc.vector.tensor_add(
                        v_loc[kk][:, n * 512 : (n + 1) * 512],
                        ps[:, :],
                        bv_sb[:, n * 512 : (n + 1) * 512],
                    )
                s = kk // 4
                nc.sync.dma_start(
                    out=agv_in[s][(kk % 4) * 128 : (kk % 4 + 1) * 128, :],
                    in_=v_loc[kk][:, :],
                )
                if kk % 4 == 3:
                    nc.gpsimd.collective_compute(
                        "AllGather",
                        mybir.AluOpType.bypass,
                        replica_groups=RG,
                        ins=[agv_in[s].opt()],
                        outs=[agv_out[s].opt()],
                    )
            vloc.release()

        kloc.release()
        wvp.release()

        # ---- Assemble-load K^T / V moved below Phase C (see there) ----

        # ---- Phase C: Q^T[c, q] = Wq^T.T @ X^T  (+bq) ----
        with nc.named_scope("proj_q"):
            for c in range(CT):
                for n in range(QLEN // 512):
                    ps = ps_s.tile([128, 512], F32, tag="ps", name="psq")
                    for d in range(DT):
                        nc.tensor.matmul(
                            ps[:, :],
                            wq_t[d][:, c * 128 : (c + 1) * 128],
                            x_t[d][:, n * 512 : (n + 1) * 512],
                            start=(d == 0),
                            stop=(d == DT - 1),
                        )
                    nc.scalar.activation(
                        q_sb[c][:, n * 512 : (n + 1) * 512],
                        ps[:, :],
                        Id,
                        bias=bq_sb[:, c : c + 1],
                    )

        wqp.release()
        xtp.release()

        # ---- Assemble K^T and V from the AllGather outputs ----
        # agk_out[s] rows: [rank0 c-tiles (keys 0:1024) | rank1 c-tiles (keys 1024:2048)]
        for c in range(CT):
            s, cc = c // 4, c % 4
            nc.sync.dma_start(
                out=k_sb[c][:, 0:QLEN],
                in_=agk_out[s][cc * 128 : (cc + 1) * 128, :],
            )
            nc.sync.dma_start(
                out=k_sb[c][:, QLEN:S],
                in_=agk_out[s][512 + cc * 128 : 512 + (cc + 1) * 128, :],
            )
        # agv_out[s] rows: [rank0 k-tiles (global k = s*4 + 0..3) | rank1 (global k = 8 + s*4 + 0..3)]
        for k in range(KT):
            h, kk = k // KTH, k % KTH
            s, r = kk // 4, kk % 4
            nc.sync.dma_start(
                out=v_sb[k][:, :],
                in_=agv_out[s][h * 512 + r * 128 : h * 512 + (r + 1) * 128, :],
            )

        # ---- Phase D/E: attention, one 512-query chunk at a time ----
        # Normalize P before the V matmul so only ONE attn@V GEMM is needed:
        #   A^T = P1^T * bcast(1/r1) - P2^T * bcast(scalar/r2);  out = A^T.T @ V
        # r_j from an ones-row stationary matmul (column sums of P^T). The
        # j=1 stationary is filled with 1/scalar so r_1' = r_1/scalar and a
        # single fast reciprocal gives bc_1 = scalar/r_1 directly.
        ones_sq = const.tile([128, 128], BF16)
        ones_sqf = const.tile([128, 128], F32)
        nc.vector.memset(ones_sqf[:, :], 1.0)
        nc.vector.tensor_copy(ones_sq[:, :], ones_sqf[:, :])
        scinv = const.tile([128, 1], F32)
        nc.vector.reciprocal(scinv[:, :], sc_sb[:, :])
        onesc_sq = const.tile([128, 128], BF16)
        nc.vector.tensor_scalar_mul(onesc_sq[:, :], ones_sqf[:, :], scinv[:, :])
        ones_j = [ones_sq, onesc_sq]

        with (
            tc.tile_pool(name="pP", bufs=2) as pP,
            tc.tile_pool(name="ps_r", bufs=1, space="PSUM") as ps_r,
            tc.tile_pool(name="ps_u", bufs=4, space="PSUM") as ps_u,
            tc.tile_pool(name="small", bufs=4) as small,
            tc.tile_pool(name="tmp2", bufs=2) as tmp2,
            tc.tile_pool(name="ostage", bufs=2) as ostage,
        ):
            for qc in range(NQC):
                # double-buffered across qc so next chunk's scores overlap
                # this chunk's combine + attn@V
                p_sb = [
                    [
                        pP.tile([128, 512], BF16, tag=f"p{j}_{k}", name=f"p{j}_{k}")
                        for k in range(KT)
                    ]
                    for j in range(2)
                ]
                # scores S^T[k, q] = K_j^T.T @ Q_j^T; P = exp(s*S^T); r = col sums
                bcs = []
                scope_s = nc.enter_named_scope(f"attn_s{qc}", False)
                for j in range(2):
                    # r replicated across partitions: ones[128,128].T @ P = col sums
                    r_ps = ps_r.tile([128, 512], F32, tag="r", name=f"r{j}")
                    for k in range(KT):
                        ps = ps_s.tile([128, 512], F32, tag="ps", name="pss")
                        for ci in range(4):
                            c = 4 * j + ci
                            nc.tensor.matmul(
                                ps[:, :],
                                k_sb[c][:, k * 128 : (k + 1) * 128],
                                q_sb[c][:, qc * 512 : (qc + 1) * 512],
                                start=(ci == 0),
                                stop=(ci == 3),
                            )
                        nc.scalar.activation(
                            p_sb[j][k][:, :], ps[:, :], Exp, scale=SCALE
                        )
                        nc.tensor.matmul(
                            r_ps[:, :],
                            ones_j[j][:, :],
                            p_sb[j][k][:, :],
                            start=(k == 0),
                            stop=(k == KT - 1),
                        )
                    # bc_j = 1/r_j' = exp(-ln r_j') on the Scalar engine (the
                    # 1/scalar factor for j=1 is folded into the rowsum
                    # stationary, so no bias term is needed)
                    lnr = tmp2.tile([128, 512], F32, tag="lnr", name="lnr")
                    nc.scalar.activation(
                        lnr[:, :], r_ps[:, :], mybir.ActivationFunctionType.Ln
                    )
                    bc = small.tile([128, 512], BF16, tag=f"bc{j}", name=f"bc{j}")
                    nc.scalar.activation(bc[:, :], lnr[:, :], Exp, scale=-1.0)
                    bcs.append(bc)
                nc.leave_named_scope(f"attn_s{qc}", scope_s[0], False)

                # Anti-throttle: a short dependent-MM chain into a corner of
                # the (now-consumed) rowsum PSUM keeps the PE HAM window busy
                # across the serial ln/exp+combine boundary so attn@V doesn't
                # start at half clock.
                for _ in range(10):
                    nc.tensor.matmul(
                        r_ps[0:2, 0:2],
                        ones_sb[:, :],
                        ones_sb[:, :],
                        start=True,
                        stop=True,
                    )

                # A^T[k] = P1[k]*bc1 - P2[k]*bc2s  (in place into p_sb[1])
                scope_a = nc.enter_named_scope(f"attn_a{qc}", False)
                for k in range(KT):
                    t2 = tmp2.tile([128, 512], BF16, tag="t2", name="t2")
                    nc.vector.tensor_mul(t2[:, :], p_sb[0][k][:, :], bcs[0][:, :])
                    nc.vector.tensor_mul(
                        p_sb[1][k][:, :], p_sb[1][k][:, :], bcs[1][:, :]
                    )
                    nc.vector.tensor_sub(p_sb[1][k][:, :], t2[:, :], p_sb[1][k][:, :])
                nc.leave_named_scope(f"attn_a{qc}", scope_a[0], False)

                # out rows = A^T.T @ V
                scope_u = nc.enter_named_scope(f"attn_u{qc}", False)
                for t in range(4):
                    row = qc * 512 + t * 128
                    for n in range(DIM // 512):
                        lo, hi = n * 512, (n + 1) * 512
                        u = ps_u.tile([128, 512], F32, tag="u", name="u")
                        for k in range(KT):
                            nc.tensor.matmul(
                                u[:, :],
                                p_sb[1][k][:, t * 128 : (t + 1) * 128],
                                v_sb[k][:, lo:hi],
                                start=(k == 0),
                                stop=(k == KT - 1),
                            )
                        o = ostage.tile([128, 512], F32, tag="o", name="o")
                        if (t * 2 + n) % 2 == 0:
                            nc.scalar.copy(o[:, :], u[:, :])
                        else:
                            nc.vector.tensor_copy(o[:, :], u[:, :])
                        nc.sync.dma_start(
                            out=outp[row : row + 128, lo:hi], in_=o[:, :]
                        )
                nc.leave_named_scope(f"attn_u{qc}", scope_u[0], False)

    return nc


_NC_CACHE = None


def _get_nc():
    global _NC_CACHE
    if _NC_CACHE is None:
        nc = _build_bass()
        fixed = _split_waits(bass.Bass.to_json_bytes(nc))
        nc.to_json_bytes = lambda: fixed
        _NC_CACHE = nc
    return _NC_CACHE


def kernel(hidden_states, W_q, b_q, W_k, b_k, W_v, b_v, scalar):
    global LAST_RESULTS
    bf16 = ml_dtypes.bfloat16
    X = np.asarray(hidden_states, np.float32)
    wqt = np.ascontiguousarray(np.asarray(W_q, np.float32).T).astype(bf16)
    wkt = np.ascontiguousarray(np.asarray(W_k, np.float32).T).astype(bf16)
    wvt = np.ascontiguousarray(np.asarray(W_v, np.float32).T).astype(bf16)
    bqr = np.ascontiguousarray(np.asarray(b_q, np.float32).reshape(CT, 128).T)
    bkr = np.ascontiguousarray(np.asarray(b_k, np.float32).reshape(CT, 128).T)
    bvb = np.ascontiguousarray(
        np.broadcast_to(np.asarray(b_v, np.float32), (128, DIM))
    )
    scv = np.full((128, 1), np.asarray(scalar, np.float32).reshape(-1)[0], np.float32)

    in_maps = []
    xts = {}
    for core in range(NCORES):
        b, h = core // 2, core % 2
        if b not in xts:
            xts[b] = np.asarray(X[b].T, np.float32)
        xth = np.ascontiguousarray(xts[b][:, h * QLEN : (h + 1) * QLEN]).astype(bf16)
        in_maps.append(
            {
                "xth": xth,
                "wqt": wqt,
                "wkt": wkt,
                "wvt": wvt,
                "bqr": bqr,
                "bkr": bkr,
                "bvb": bvb,
                "scv": scv,
            }
        )

    nc = _get_nc()
    res = run_bass_kernel_spmd(
        nc,
        in_maps,
        list(range(NCORES)),
        trace=TRACE,
    )
    LAST_RESULTS = res

    out = np.empty((B, S, DIM), np.float32)
    for core in range(NCORES):
        b, h = core // 2, core % 2
        out[b, h * QLEN : (h + 1) * QLEN, :] = res.results[core]["out"]
    return out


if __name__ == "__main__":
    import reference

    inputs = {k: np.asarray(v) for k, v in reference.setup_inputs().items()}
    got = kernel(**inputs)
    print("kernel output", got.shape, got.dtype)
